# revision 1
# baseline (speedup 1.0000x reference)

# Trainium2 Bass kernel for nn_AttentionGeo (gnn_message_passing).
#
# Math (per point b of B=32768, K=50 neighbors, F=80 context feats, D=64):
#   n2v1 = mlp1(node2vec)          [B, K+1, 64]   (only row 0 used)
#   n2v2 = mlp2(node2vec)          [B, K+1, 64]   (only rows 1..K used)
#   target  = l2norm(n2v1[:, 0])   [B, 64]
#   neighbor= l2norm(n2v2[:, 1:])  [B, K, 64]
#   simi = exp(-d^2) + 0.1 * mean(target*neighbor, -1)
#   weight = softmax(simi @ kernel + bias)
#   out = einsum('bk,bkf->bf', weight, context)
#
# Strategy: pure data-parallel over 8 cores (4096 points each), tiles of 128
# points. Within a tile, 26 "k-pair blocks": partitions 0-63 hold features of
# the even-slot k, 64-127 the odd-slot k; block 0 holds the target (k=0)
# duplicated in both halves. MLP runs feature-major via PE transposes
# (data-as-stationary matmul against identity) and concurrent quadrant
# matmuls. Dot products and sum-of-squares reduce over features via the
# stationary-operand matmul trick (product block as lhsT, 2-column ones mask
# as rhs -> point-major [128, 2] PSUM outputs). l2norm uses exp(-0.5*ln(x))
# so a single ACT table set (natural_log_exp_and_others) covers everything.

import math

import numpy as np

B, K, F, D = 32768, 50, 80, 64
NCORES = 8
BC = B // NCORES            # points per core
P = 128                     # partitions / points per tile
NBLK = (K + 2) // 2         # 26 k-pair blocks (block 0 = target twice)
COLS = NBLK * P             # 3328 packed columns per tile
CHUNK_BLKS = 4              # blocks per psum chunk (512 cols)

_CACHE = {}


def _build(nc, bc, mybir, tile_mod, reps=1, x_cast_dma=True,
           do_mlp=True, do_agg=True, ps_cfg=(2, 0, 2, 2, -2),
           tmode=True):
    # ps_cfg = (xt, hy_shared, h, y, a+c) buf counts; hy_shared>0 overrides h/y
    xt_b, hy_b, h_b, y_b, ac_b = ps_cfg
    fp32 = mybir.dt.float32
    bf16 = mybir.dt.bfloat16
    AF = mybir.ActivationFunctionType
    OP = mybir.AluOpType
    TileContext = tile_mod.TileContext

    nt = bc // P

    # ---- DRAM I/O ------------------------------------------------------
    n2v = nc.dram_tensor("n2v", [bc, 51, D], fp32, kind="ExternalInput").ap()
    ctx_d = nc.dram_tensor("ctx", [bc, K, F], fp32, kind="ExternalInput").ap()
    dist = nc.dram_tensor("dist", [bc, K], fp32, kind="ExternalInput").ap()
    out_d = nc.dram_tensor("out", [bc, F], fp32, kind="ExternalOutput").ap()

    # tiny replicated constants (prepared host-side)
    ident_bf_d = nc.dram_tensor("ident_bf", [P, P], bf16, kind="ExternalInput").ap()
    ident_f32_d = nc.dram_tensor("ident_f32", [P, P], fp32, kind="ExternalInput").ap()
    ones2_d = nc.dram_tensor("ones2", [P, 2], bf16, kind="ExternalInput").ap()
    w1d1_d = nc.dram_tensor("w1d1", [P, D], bf16, kind="ExternalInput").ap()
    w1d2_d = nc.dram_tensor("w1d2", [P, D], bf16, kind="ExternalInput").ap()
    w2d1_d = nc.dram_tensor("w2d1", [P, D], bf16, kind="ExternalInput").ap()
    w2d2_d = nc.dram_tensor("w2d2", [P, D], bf16, kind="ExternalInput").ap()
    w2T_d = nc.dram_tensor("w2T", [P, D], bf16, kind="ExternalInput").ap()
    b2half_d = nc.dram_tensor("b2half", [P, 1], bf16, kind="ExternalInput").ap()
    b1d1_d = nc.dram_tensor("b1d1", [P, 1], fp32, kind="ExternalInput").ap()
    b1d2_d = nc.dram_tensor("b1d2", [P, 1], fp32, kind="ExternalInput").ap()
    b2d1_d = nc.dram_tensor("b2d1", [P, 1], fp32, kind="ExternalInput").ap()
    b2d2_d = nc.dram_tensor("b2d2", [P, 1], fp32, kind="ExternalInput").ap()
    bias_bc_d = nc.dram_tensor("bias_bc", [P, K], fp32, kind="ExternalInput").ap()
    kern_d = nc.dram_tensor("kern", [K, K], fp32, kind="ExternalInput").ap()

    LN_C = math.log(0.1 / 64.0)  # folds the 0.1 coeff and the mean's 1/64

    from contextlib import ExitStack

    with TileContext(nc) as tc, ExitStack() as es:
        const = es.enter_context(tc.tile_pool(name="const", bufs=1))
        io = es.enter_context(tc.tile_pool(name="io", bufs=2))
        io3 = es.enter_context(tc.tile_pool(name="io3", bufs=3))
        mid = es.enter_context(tc.tile_pool(name="mid", bufs=2))
        small = es.enter_context(tc.tile_pool(name="small", bufs=4))
        ps_xt_pool = es.enter_context(
            tc.tile_pool(name="ps_xt", bufs=xt_b, space="PSUM"))
        if hy_b:
            ps_h_pool = ps_y_pool = es.enter_context(
                tc.tile_pool(name="ps_hy", bufs=hy_b, space="PSUM"))
        else:
            ps_h_pool = es.enter_context(
                tc.tile_pool(name="ps_h", bufs=h_b, space="PSUM"))
            ps_y_pool = es.enter_context(
                tc.tile_pool(name="ps_y", bufs=y_b, space="PSUM"))
        if ac_b < 0:
            ps_a_pool = ps_c_pool = es.enter_context(
                tc.tile_pool(name="ps_m", bufs=-ac_b, space="PSUM"))
        else:
            ps_a_pool = es.enter_context(
                tc.tile_pool(name="ps_a", bufs=ac_b, space="PSUM"))
            ps_c_pool = es.enter_context(
                tc.tile_pool(name="ps_c", bufs=ac_b, space="PSUM"))

        def cload(dram_ap, shape, dtype, tag):
            t = const.tile(shape, dtype, tag=tag)
            nc.sync.dma_start(out=t, in_=dram_ap)
            return t

        ident_bf = cload(ident_bf_d, [P, P], bf16, "ident_bf")
        ident_f32 = cload(ident_f32_d, [P, P], fp32, "ident_f32")
        ones2 = cload(ones2_d, [P, 2], bf16, "ones2")
        w1d1 = cload(w1d1_d, [P, D], bf16, "w1d1")
        w1d2 = cload(w1d2_d, [P, D], bf16, "w1d2")
        w2d1 = cload(w2d1_d, [P, D], bf16, "w2d1")
        w2d2 = cload(w2d2_d, [P, D], bf16, "w2d2")
        w2T = cload(w2T_d, [P, D], bf16, "w2T")
        b2half = cload(b2half_d, [P, 1], bf16, "b2half")
        b1d1 = cload(b1d1_d, [P, 1], fp32, "b1d1")
        b1d2 = cload(b1d2_d, [P, 1], fp32, "b1d2")
        b2d1 = cload(b2d1_d, [P, 1], fp32, "b2d1")
        b2d2 = cload(b2d2_d, [P, 1], fp32, "b2d2")
        bias_bc = cload(bias_bc_d, [P, K], fp32, "bias_bc")
        kern = cload(kern_d, [K, K], fp32, "kern")

        zbias = const.tile([P, 1], fp32, tag="zbias")
        nc.gpsimd.memset(zbias, 0.0)
        magic = const.tile([P, 1], mybir.dt.int32, tag="magic")
        nc.gpsimd.memset(magic, 0x5F3759DF)

        n2v_f = n2v.rearrange("b k f -> b (k f)")
        ctx_f = ctx_d.rearrange("b k f -> b (k f)")

        def issue_loads(t):
            """Input DMAs for tile t (issued one tile ahead so the SWDGE
            descriptor generation isn't queued behind the previous tile's
            GPSIMD aggregation multiply)."""
            rows = slice(t * P, (t + 1) * P)
            if x_cast_dma:
                x_bf = io.tile([P, 51 * D], bf16, tag="x")  # cast on load
                nc.gpsimd.dma_start(out=x_bf, in_=n2v_f[rows])
            else:
                x_bf = io.tile([P, 51 * D], fp32, tag="x")
                nc.sync.dma_start(out=x_bf, in_=n2v_f[rows])
            ctx_sb = io3.tile([P, K * F], fp32, tag="ctx")
            nc.sync.dma_start(out=ctx_sb, in_=ctx_f[rows])
            d_sb = io3.tile([P, K], fp32, tag="d")
            nc.sync.dma_start(out=d_sb, in_=dist[rows])
            return x_bf, ctx_sb, d_sb

        from contextlib import nullcontext

        def stage1(t, x_bf, ctx_sb, d_sb):
            """Loads -> transposes -> MLP -> feature reductions (PE-heavy).
            Emitted one tile AHEAD of stage2 so the PE stream never stalls
            on the previous tile's similarity/softmax tail."""
            x_v = x_bf.rearrange("p (k f) -> p k f", f=D)

            xT = mid.tile([P, COLS], bf16, tag="xT")
            h = mid.tile([P, COLS], bf16, tag="h")
            y2 = mid.tile([P, COLS], bf16, tag="y2")
            uh = mid.tile([P, COLS], bf16, tag="uh")
            yt_sb = mid.tile([P, P], bf16, tag="yt")
            v_sb = mid.tile([P, P], bf16, tag="v")

            # one psum bank per tile for RED outputs + v + dotb2 + the later
            # simiT/logits (single PE-write era, then reads, then tail MMs)
            pma = ps_a_pool.tile([P, 512], fp32, tag="pma")
            pm_sumsq = pma[:, 0:52]
            pm_dots = pma[:, 64:116]
            pm_v = pma[:, 128:256]
            pm_dotb2 = pma[:, 256:257]

            nchunks = (NBLK + CHUNK_BLKS - 1) // CHUNK_BLKS if do_mlp else 0
            for c in range(nchunks):
                b0 = c * CHUNK_BLKS
                nb = min(CHUNK_BLKS, NBLK - b0)
                ncol = nb * P
                cs = slice(b0 * P, b0 * P + ncol)

                # ---- transposes (transpose-mode: bf16 psum, 2x evac) ----
                xt_dt = bf16 if (x_cast_dma and tmode) else fp32
                ident_x = ident_bf if x_cast_dma else ident_f32
                ps_xt = ps_xt_pool.tile([P, 512], xt_dt, tag="psxt")
                for j in range(nb):
                    blk = b0 + j
                    if blk == 0:
                        x0 = x_bf[:, 0:D]
                        if tmode:
                            nc.tensor.transpose(ps_xt[0:64, 0:P], x0, ident_x,
                                                tile_position=(0, 0))
                            nc.tensor.transpose(ps_xt[64:128, 0:P], x0,
                                                ident_x, tile_position=(0, 64))
                        else:
                            nc.tensor.matmul(ps_xt[0:64, 0:P], x0, ident_x,
                                             start=True, stop=True,
                                             tile_position=(0, 0))
                            nc.tensor.matmul(ps_xt[64:128, 0:P], x0, ident_x,
                                             start=True, stop=True,
                                             tile_position=(0, 64))
                    else:
                        xpair = x_bf[:, (2 * blk - 1) * D:(2 * blk + 1) * D]
                        if tmode:
                            nc.tensor.transpose(ps_xt[:, j * P:(j + 1) * P],
                                                xpair, ident_x)
                        else:
                            nc.tensor.matmul(ps_xt[:, j * P:(j + 1) * P],
                                             xpair, ident_x, start=True,
                                             stop=True)
                nc.any.tensor_copy(xT[:, cs], ps_xt[:, 0:ncol])

                # ---- MLP layer 1 (quadrant matmuls) ----
                ps_h = ps_h_pool.tile([P, 512], fp32, tag="psh")
                xT_c = xT[:, cs]
                if c == 0:
                    nc.tensor.matmul(ps_h[0:64, 0:P], w1d1[0:64, :],
                                     xT_c[0:64, 0:P], start=True, stop=True,
                                     tile_position=(0, 0))
                    nc.tensor.matmul(ps_h[0:64, P:ncol], w1d2[0:64, :],
                                     xT_c[0:64, P:ncol], start=True, stop=True,
                                     tile_position=(0, 0))
                    nc.tensor.matmul(ps_h[64:128, 0:P], w1d1[64:128, :],
                                     xT_c[64:128, 0:P], start=True, stop=True,
                                     tile_position=(64, 64))
                    nc.tensor.matmul(ps_h[64:128, P:ncol], w1d2[64:128, :],
                                     xT_c[64:128, P:ncol], start=True,
                                     stop=True, tile_position=(64, 64))
                else:
                    nc.tensor.matmul(ps_h[0:64, 0:ncol], w1d2[0:64, :],
                                     xT_c[0:64, :], start=True, stop=True,
                                     tile_position=(0, 0))
                    nc.tensor.matmul(ps_h[64:128, 0:ncol], w1d2[64:128, :],
                                     xT_c[64:128, :], start=True, stop=True,
                                     tile_position=(64, 64))

                # ---- relu + bias evac (alternate DVE / ACT) ----
                def relu_evac(dst, src, bias_ap, use_dve):
                    if use_dve:
                        nc.vector.tensor_scalar(dst, src, bias_ap, 0.0,
                                                OP.add, OP.max)
                    else:
                        nc.scalar.activation(dst, src, AF.Relu, bias=bias_ap)

                if c == 0:
                    relu_evac(h[:, 0:P], ps_h[:, 0:P], b1d1, False)
                    relu_evac(h[:, P:ncol], ps_h[:, P:ncol], b1d2, c % 2 == 1)
                else:
                    relu_evac(h[:, cs], ps_h[:, 0:ncol], b1d2, c % 2 == 1)

                # ---- MLP layer 2 ----
                ps_y = ps_y_pool.tile([P, 512], fp32, tag="psy")
                h_c = h[:, cs]
                if c == 0:
                    nc.tensor.matmul(ps_y[0:64, 0:P], w2d1[0:64, :],
                                     h_c[0:64, 0:P], start=True, stop=True,
                                     tile_position=(0, 0))
                    nc.tensor.matmul(ps_y[0:64, P:ncol], w2d2[0:64, :],
                                     h_c[0:64, P:ncol], start=True, stop=True,
                                     tile_position=(0, 0))
                    nc.tensor.matmul(ps_y[64:128, 0:P], w2d1[64:128, :],
                                     h_c[64:128, 0:P], start=True, stop=True,
                                     tile_position=(64, 64))
                    nc.tensor.matmul(ps_y[64:128, P:ncol], w2d2[64:128, :],
                                     h_c[64:128, P:ncol], start=True,
                                     stop=True, tile_position=(64, 64))
                else:
                    nc.tensor.matmul(ps_y[0:64, 0:ncol], w2d2[0:64, :],
                                     h_c[0:64, :], start=True, stop=True,
                                     tile_position=(0, 0))
                    nc.tensor.matmul(ps_y[64:128, 0:ncol], w2d2[64:128, :],
                                     h_c[64:128, :], start=True, stop=True,
                                     tile_position=(64, 64))

                # ---- y^2 evac (squared MLP2 output, +bias, via ACT) ----
                if c == 0:
                    nc.scalar.activation(y2[:, 0:P], ps_y[:, 0:P], AF.Square,
                                         bias=b2d1)
                    nc.scalar.activation(y2[:, P:ncol], ps_y[:, P:ncol],
                                         AF.Square, bias=b2d2)
                    # target row (feature-major, both halves) + its bias
                    nc.vector.tensor_scalar(yt_sb, ps_y[:, 0:P], b2d1, None,
                                            OP.add)
                    # v = W2_d2^T-contract with yt (per-point, both halves)
                    nc.tensor.matmul(pm_v[0:64, :], w2T[0:64, :],
                                     yt_sb[0:64, :], start=True, stop=True,
                                     tile_position=(0, 0))
                    nc.tensor.matmul(pm_v[64:128, :], w2T[64:128, :],
                                     yt_sb[64:128, :], start=True, stop=True,
                                     tile_position=(64, 64))
                    nc.any.tensor_copy(v_sb, pm_v)
                    # dotb2[b] = yt[b] . d2_b2
                    nc.tensor.matmul(pm_dotb2, yt_sb, b2half, start=True,
                                     stop=True)
                else:
                    nc.scalar.activation(y2[:, cs], ps_y[:, 0:ncol],
                                         AF.Square, bias=b2d2)

                # ---- uh = h * v (per-point broadcast over blocks) ----
                h3 = h[:, cs].rearrange("p (a q) -> p a q", q=P)
                uh3 = uh[:, cs].rearrange("p (a q) -> p a q", q=P)
                vb = v_sb.unsqueeze(1).broadcast_to([P, nb, P])
                nc.vector.tensor_tensor(uh3, h3, vb, OP.mult)

                # ---- feature reductions via stationary-matmul ----
                for j in range(nb):
                    blk = b0 + j
                    bs = slice(blk * P, (blk + 1) * P)
                    nc.tensor.matmul(pm_sumsq[:, 2 * blk:2 * blk + 2],
                                     y2[:, bs], ones2, start=True, stop=True)
                    if blk > 0:
                        nc.tensor.matmul(pm_dots[:, 2 * blk:2 * blk + 2],
                                         uh[:, bs], ones2, start=True,
                                         stop=True)

            return dict(ctx_sb=ctx_sb, d_sb=d_sb, pma=pma)

        def stage2(t, st):
            """Similarity, softmax, context aggregation for tile t (emitted
            during tile t+1's stage1)."""
            rows = slice(t * P, (t + 1) * P)
            ctx_sb, d_sb, pma = st["ctx_sb"], st["d_sb"], st["pma"]
            pm_sumsq = pma[:, 0:52]
            pm_dots = pma[:, 64:116]
            pm_dotb2 = pma[:, 256:257]
            pm_simiT = pma[0:K, 288:416]
            pm_logits = pma[:, 416:416 + K]

            if not do_mlp:
                dsq0 = small.tile([P, K], fp32, tag="dsq0")
                nc.vector.tensor_tensor(dsq0, d_sb, d_sb, OP.mult)
                simi = small.tile([P, K], fp32, tag="simi")
                nc.scalar.activation(simi, dsq0, AF.Exp, scale=-1.0,
                                     bias=zbias)
            else:
                # s = rsqrt(St * Sn) via bit-trick seed + 2 Newton steps (DVE
                # only -- keeps every ACT func in exp_and_others).
                St_sb = small.tile([P, 1], fp32, tag="St")
                nc.vector.tensor_copy(St_sb, pm_sumsq[:, 0:1])
                q_sc = small.tile([P, K], fp32, tag="q")
                nc.vector.tensor_tensor(q_sc, pm_sumsq[:, 2:52],
                                        St_sb.broadcast_to([P, K]), OP.mult)
                sh_i = small.tile([P, K], mybir.dt.int32, tag="sh")
                nc.vector.tensor_scalar(sh_i, q_sc.bitcast(mybir.dt.int32),
                                        1, None, OP.logical_shift_right)
                x0_i = small.tile([P, K], mybir.dt.int32, tag="x0")
                nc.vector.tensor_tensor(
                    x0_i,
                    magic.broadcast_to([P, K]).bitcast(mybir.dt.int32),
                    sh_i, OP.subtract)
                x_nr = x0_i.bitcast(fp32)
                for it in range(2):
                    aa = small.tile([P, K], fp32, tag=f"nr_a{it}")
                    nc.vector.tensor_tensor(aa, x_nr, x_nr, OP.mult)
                    bb = small.tile([P, K], fp32, tag=f"nr_b{it}")
                    nc.vector.tensor_tensor(bb, q_sc, aa, OP.mult)
                    cc = small.tile([P, K], fp32, tag=f"nr_c{it}")
                    nc.vector.tensor_scalar(cc, bb, -0.5, 1.5, OP.mult,
                                            OP.add)
                    xn = small.tile([P, K], fp32, tag=f"nr_x{it}")
                    nc.vector.tensor_tensor(xn, x_nr, cc, OP.mult)
                    x_nr = xn

                # D = (raw_dots + dotb2) * rsqrt (0.1/64 pre-folded in w2T)
                D_sb = small.tile([P, K], fp32, tag="D")
                nc.vector.scalar_tensor_tensor(D_sb, pm_dots[:, 2:52],
                                               pm_dotb2, x_nr, OP.add,
                                               OP.mult)
                # simi1 = exp(-d^2)
                dsq = small.tile([P, K], fp32, tag="dsq")
                nc.vector.tensor_tensor(dsq, d_sb, d_sb, OP.mult)
                simi1 = small.tile([P, K], fp32, tag="simi1")
                nc.scalar.activation(simi1, dsq, AF.Exp, scale=-1.0,
                                     bias=zbias)
                simi = small.tile([P, K], fp32, tag="simi")
                nc.vector.tensor_tensor(simi, simi1, D_sb, OP.add)

            # ---- logits = simi @ kernel + bias ----
            nc.tensor.matmul(pm_simiT, simi, ident_f32, start=True, stop=True)
            simiT_sb = small.tile([K, P], fp32, tag="simiT")
            nc.any.tensor_copy(simiT_sb, pm_simiT)
            nc.tensor.matmul(pm_logits, simiT_sb, kern, start=True, stop=True)
            logits = small.tile([P, K], fp32, tag="logits")
            nc.vector.tensor_tensor(logits, pm_logits, bias_bc, OP.add)

            # ---- softmax over k (no max-subtraction; 1/sum applied after
            # the aggregation so the multiply starts straight off the exp) --
            e_sb = small.tile([P, K], fp32, tag="e")
            nc.scalar.activation(e_sb, logits, AF.Exp, bias=zbias)
            ssum = small.tile([P, 1], fp32, tag="ssum")
            nc.vector.tensor_reduce(ssum, e_sb, mybir.AxisListType.X, OP.add)
            rr = small.tile([P, 1], fp32, tag="rr")
            nc.vector.reciprocal(rr, ssum)

            # ---- context aggregation in k-slices (GPSIMD multiply and DVE
            # reduce pipeline against each other) ----
            if not do_agg:
                out_sb = io.tile([P, F], fp32, tag="out")
                nc.vector.tensor_tensor(out_sb, ctx_sb[:, 0:F],
                                        ctx_sb[:, F:2 * F], OP.add)
                nc.sync.dma_start(out=out_d[rows], in_=out_sb)
                return
            ctx3 = ctx_sb.rearrange("p (k f) -> p k f", f=F)
            KQ = [13, 13, 12, 12]
            parts = []
            k0 = 0
            for hx, kq in enumerate(KQ):
                ks = slice(k0, k0 + kq)
                k0 += kq
                prod = mid.tile([P, kq, F], fp32, tag=f"prod{hx}")
                wb = e_sb[:, ks].unsqueeze(2).broadcast_to([P, kq, F])
                nc.gpsimd.tensor_tensor(prod, ctx3[:, ks, :], wb, OP.mult)
                oh = small.tile([P, F], fp32, tag=f"outh{hx}")
                nc.vector.tensor_reduce(oh, prod.transpose([0, 2, 1]),
                                        mybir.AxisListType.X, OP.add)
                parts.append(oh)
            o01 = small.tile([P, F], fp32, tag="o01")
            nc.vector.tensor_tensor(o01, parts[0], parts[1], OP.add)
            o23 = small.tile([P, F], fp32, tag="o23")
            nc.vector.tensor_tensor(o23, parts[2], parts[3], OP.add)
            osum = small.tile([P, F], fp32, tag="osum")
            nc.vector.tensor_tensor(osum, o01, o23, OP.add)
            out_sb = io.tile([P, F], fp32, tag="out")
            nc.vector.tensor_scalar(out_sb, osum, rr, None, OP.mult)
            nc.sync.dma_start(out=out_d[rows], in_=out_sb)

        rep_cm = tc.For_i(0, reps, 1) if reps > 1 else nullcontext()
        with rep_cm:
          pending = issue_loads(0)
          states = {}
          for it in range(nt + 1):
              if it < nt:
                  x_bf, ctx_sb, d_sb = pending
                  if it + 1 < nt:
                      pending = issue_loads(it + 1)
                  states[it] = stage1(it, x_bf, ctx_sb, d_sb)
              if it >= 1:
                  stage2(it - 1, states.pop(it - 1))

    return nc


def _make_consts():
    """Host-side constant blocks shared by every core (keyed by input name)."""
    return None  # filled in _prep_inputs


def _prep_inputs(inputs):
    f32 = np.float32
    import ml_dtypes
    bf16 = ml_dtypes.bfloat16

    d1_w1 = inputs["d1_w1"].astype(f32)
    d1_w2 = inputs["d1_w2"].astype(f32)
    d2_w1 = inputs["d2_w1"].astype(f32)
    d2_w2 = inputs["d2_w2"].astype(f32)
    d1_b1 = inputs["d1_b1"].astype(f32)
    d1_b2 = inputs["d1_b2"].astype(f32)
    d2_b1 = inputs["d2_b1"].astype(f32)
    d2_b2 = inputs["d2_b2"].astype(f32)

    consts = {
        "ident_bf": np.eye(P, dtype=bf16),
        "ident_f32": np.eye(P, dtype=f32),
        "ones2": np.concatenate(
            [np.repeat([[1, 0]], 64, 0), np.repeat([[0, 1]], 64, 0)]
        ).astype(bf16),
        "w1d1": np.vstack([d1_w1, d1_w1]).astype(bf16),
        "w1d2": np.vstack([d2_w1, d2_w1]).astype(bf16),
        "w2d1": np.vstack([d1_w2, d1_w2]).astype(bf16),
        "w2d2": np.vstack([d2_w2, d2_w2]).astype(bf16),
        # 0.1 coeff and the mean's 1/64 are folded into the dot-product path
        "w2T": (np.vstack([d2_w2.T, d2_w2.T]) * (0.1 / 64.0)).astype(bf16),
        "b2half": (np.concatenate([d2_b2, np.zeros(64, f32)])[:, None]
                   * (0.1 / 64.0)).astype(bf16),
        "b1d1": np.concatenate([d1_b1, d1_b1])[:, None].astype(f32),
        "b1d2": np.concatenate([d2_b1, d2_b1])[:, None].astype(f32),
        "b2d1": np.concatenate([d1_b2, d1_b2])[:, None].astype(f32),
        "b2d2": np.concatenate([d2_b2, d2_b2])[:, None].astype(f32),
        "bias_bc": np.tile(inputs["bias"].astype(f32)[None, :], (P, 1)),
        "kern": inputs["kernel"].astype(f32),
    }

    n2v = np.ascontiguousarray(inputs["node2vec"].astype(f32))
    ctx = np.ascontiguousarray(inputs["context"].astype(f32))
    dist = np.ascontiguousarray(inputs["source_distance"].astype(f32))

    in_maps = []
    for c in range(NCORES):
        sl = slice(c * BC, (c + 1) * BC)
        m = dict(consts)
        m["n2v"] = n2v[sl]
        m["ctx"] = ctx[sl]
        m["dist"] = dist[sl]
        in_maps.append(m)
    return in_maps


def build(bc=BC, reps=1, **kw):
    import concourse.mybir as mybir
    import concourse.tile as tile_mod
    from concourse import bacc

    nc = bacc.Bacc("TRN2", target_bir_lowering=False, debug=False,
                   num_devices=NCORES)
    _build(nc, bc, mybir, tile_mod, reps=reps, **kw)
    nc.finalize()
    return nc


def kernel(**inputs):
    from concourse import bass_utils

    if "nc" not in _CACHE:
        _CACHE["nc"] = build(BC)
    nc = _CACHE["nc"]
    in_maps = _prep_inputs(inputs)
    res = bass_utils.run_bass_kernel_spmd(nc, in_maps,
                                          core_ids=list(range(NCORES)))
    out = np.concatenate([r["out"] for r in res.results], axis=0)
    return out.astype(np.float32)

